# revision 26
# baseline (speedup 1.0000x reference)
"""AdaptiveInput embedding lookup kernel for TRN2 (8 NeuronCores).

Strategy: pure data-parallel over tokens. tokens (8, 4096) -> one batch row
per core (4096 tokens each); embedding tables replicated to every core; no
collectives.

The three cluster tables are concatenated host-side into one flat f32 buffer
viewed as [80263, 256] (head rows start at window-row 0, tail0 rows at 20000,
tail1 at 50000, 3 zero rows of padding).  Every token gathers one uniform
4KB window starting at its table row, so a single indirect-DMA gather stream
serves all three clusters, and the window's first 1024|512|256 floats are
exactly the token's embedding row.

Per-core layout: token j sits at (partition p=j%128, slot s=j//128); 4 chunks
of 8 slots (1024 tokens).  Per chunk:
  * per-slot [128,1]-offset indirect_dma_start gathers windows into the
    output tile W [128, 8, 1024] f32
  * per slot: mask-multiply-cast W[:, s, :512]/[:, s, :256] to bf16 (zeroing
    rows of tokens from other clusters), PE-transpose 128x128 blocks to put
    the contraction dim on partitions, matmul against pre-transposed bf16
    weights into PSUM, then copy_predicated the projection into W for every
    non-head token (head rows keep the gathered embedding)
  * DMA W to HBM rows s*128+p
"""

import numpy as np

import concourse.bass as bass
import concourse.mybir as mybir
import concourse.tile as tile
from concourse import bacc
from concourse.bass import IndirectOffsetOnAxis
from concourse.masks import make_identity

FP32 = mybir.dt.float32
BF16 = mybir.dt.bfloat16
I32 = mybir.dt.int32
Alu = mybir.AluOpType

P = 128
D = 1024  # IN_FEATURES
HEAD_ROWS = 5000
T0_ROWS, T0_H = 15000, 512
T1_ROWS, T1_H = 30257, 256
CUT1, CUT2 = 5000, 20000

# flat concat of tables in 256-float windows rows
WROW = 256
T0_BASE = HEAD_ROWS * (D // WROW)            # 20000
T1_BASE = T0_BASE + T0_ROWS * (T0_H // WROW)  # 50000
TAB_ROWS = T1_BASE + T1_ROWS * (T1_H // WROW) + 3  # 80260 + 3 pad rows

N_CORES = 8


def build_graph(n_tok=4096, chunk_slots=8):
    n_slots = n_tok // P
    n_chunks = n_slots // chunk_slots
    assert n_chunks * chunk_slots == n_slots

    nc = bacc.Bacc("TRN2", target_bir_lowering=False, debug=False)

    tok_ext = nc.dram_tensor("tokens", [n_tok], I32, kind="ExternalInput")
    tab_ext = nc.dram_tensor("tables", [TAB_ROWS, WROW], FP32, kind="ExternalInput")
    t0wT_ext = nc.dram_tensor("tail0_wT", [P, T0_H // P, D], BF16,
                              kind="ExternalInput")
    t1wT_ext = nc.dram_tensor("tail1_wT", [P, T1_H // P, D], BF16,
                              kind="ExternalInput")
    out_ext = nc.dram_tensor("out", [n_tok, D], FP32, kind="ExternalOutput")
    out_r = out_ext.rearrange("(p s) d -> p s d", p=P)  # row p*n_slots+s

    with tile.TileContext(nc) as tc:
        with (
            tc.tile_pool(name="const", bufs=1) as cpool,
            tc.tile_pool(name="work", bufs=1) as wpool,
            tc.tile_pool(name="eT", bufs=3) as etpool,
            tc.tile_pool(name="psT0", bufs=3, space="PSUM") as psT0pool,
            tc.tile_pool(name="psMM", bufs=2, space="PSUM") as psMMpool,
        ):
            # ---- token-derived window indices and masks (first: the
            # gathers depend on these) ----
            # token j at (p=j//n_slots, s=j%n_slots): contiguous DMA
            tok_m = cpool.tile([P, n_slots], I32, tag="tok_m")
            nc.sync.dma_start(tok_m[:], tok_ext.rearrange("(p s) -> p s", p=P))

            v = nc.vector

            ge5k = cpool.tile([P, n_slots], I32, tag="ge5k")
            tmp = cpool.tile([P, n_slots], I32, tag="tmpi")
            tmp2 = cpool.tile([P, n_slots], I32, tag="tmpi2")
            widx = cpool.tile([P, n_slots], I32, tag="widx")
            m1b = cpool.tile([P, n_slots], BF16, tag="m1b")
            m2b = cpool.tile([P, n_slots], BF16, tag="m2b")

            v.tensor_scalar(ge5k[:], tok_m[:], CUT1, None, op0=Alu.is_ge)
            # window row index:
            #   t<5k: 4t ; 5k<=t<20k: 2t+10000 ; t>=20k: t+30000
            # = 4t - ge5k*(2t-10000) - ge20k*(t-20000)
            v.tensor_scalar(tmp[:], tok_m[:], 2, 10000, op0=Alu.mult, op1=Alu.subtract)
            v.tensor_tensor(tmp[:], tmp[:], ge5k[:], op=Alu.mult)
            v.tensor_scalar(widx[:], tok_m[:], 4, None, op0=Alu.mult)
            v.tensor_tensor(widx[:], widx[:], tmp[:], op=Alu.subtract)
            v.tensor_scalar(tmp2[:], tok_m[:], CUT2, None, op0=Alu.subtract)
            v.tensor_scalar(tmp[:], tok_m[:], CUT2, None, op0=Alu.is_ge)
            v.tensor_copy(m2b[:], tmp[:])
            v.tensor_tensor(tmp2[:], tmp2[:], tmp[:], op=Alu.mult)
            v.tensor_tensor(widx[:], widx[:], tmp2[:], op=Alu.subtract)
            # masks: m1 = (t>=5000)&(t<20000) ; m2 = t>=20000
            v.tensor_scalar(tmp[:], tok_m[:], CUT2, None, op0=Alu.is_lt)
            v.tensor_tensor(tmp[:], tmp[:], ge5k[:], op=Alu.mult)
            v.tensor_copy(m1b[:], tmp[:])
            # head top-up window index: 4t+2 for head tokens, OOB otherwise
            idx_top = cpool.tile([P, n_slots], I32, tag="idx_top")
            v.tensor_scalar(tmp[:], ge5k[:], 100000, None, op0=Alu.mult)
            v.tensor_scalar(idx_top[:], tok_m[:], 4, 2, op0=Alu.mult, op1=Alu.add)
            v.tensor_tensor(idx_top[:], idx_top[:], tmp[:], op=Alu.add)

            # ---- constants / one-time prep ----
            ident = cpool.tile([P, P], BF16, tag="ident")
            make_identity(nc, ident[:])

            # HAM warm-up: dependency-free matmuls so the PE clock reaches
            # 8/8 before the first real transposes/matmuls arrive
            warm = cpool.tile([P, 512], BF16, tag="warm")
            nc.vector.memset(warm[:], 0.0)
            wps = psMMpool.tile([P, D], FP32, tag="mm", name="warmps")
            for _ in range(16):
                nc.tensor.matmul(out=wps[:, 0:512], lhsT=warm[:, 0:P], rhs=warm[:],
                                 start=True, stop=True)

            # weights arrive pre-transposed/pre-cast: [k%128, k//128, f] bf16
            n_k0 = T0_H // P  # 4
            n_k1 = T1_H // P  # 2
            w0T = cpool.tile([P, n_k0, D], BF16, tag="w0T")
            w1T = cpool.tile([P, n_k1, D], BF16, tag="w1T")
            nc.sync.dma_start(w0T[:], t0wT_ext[:, :, :])
            nc.sync.dma_start(w1T[:], t1wT_ext[:, :, :])

            # ---- main loop: flat slot stream, software-pipelined so the
            # PE runs transposes(s) back-to-back with matmuls(s-1) while the
            # ACT copy of slot s proceeds in parallel ----
            NBUF = 24  # slot tiles in flight (gathers run ahead)
            Wt = [wpool.tile([P, D], FP32, tag=f"W_{i}", name=f"W_{i}")
                  for i in range(NBUF)]
            e0t = [wpool.tile([P, T0_H], BF16, tag=f"e0b_{i}", name=f"e0b_{i}")
                   for i in range(NBUF)]
            e1t = [wpool.tile([P, T1_H], BF16, tag=f"e1b_{i}", name=f"e1b_{i}")
                   for i in range(NBUF)]

            def stage_front(sg):
                """gather + mask/cast + transpose + psum->sbuf copy"""
                bi = sg % NBUF
                Ws = Wt[bi]
                nc.gpsimd.indirect_dma_start(
                    out=Ws[:], out_offset=None,
                    in_=tab_ext[:, :],
                    in_offset=IndirectOffsetOnAxis(ap=widx[:, sg:sg + 1], axis=0),
                )
                v.tensor_tensor(
                    e0t[bi][:], Ws[:, 0:T0_H],
                    m1b[:, sg:sg + 1].to_broadcast([P, T0_H]), op=Alu.mult,
                )
                v.tensor_tensor(
                    e1t[bi][:], Ws[:, 0:T1_H],
                    m2b[:, sg:sg + 1].to_broadcast([P, T1_H]), op=Alu.mult,
                )
                psT = psT0pool.tile([P, n_k0 + n_k1, P], BF16, tag="psw",
                                    name=f"psT_{sg}")
                for kc in range(n_k0):
                    nc.tensor.transpose(
                        out=psT[:, kc, :],
                        in_=e0t[bi][:, kc * P:(kc + 1) * P],
                        identity=ident[:],
                    )
                for kc in range(n_k1):
                    nc.tensor.transpose(
                        out=psT[:, n_k0 + kc, :],
                        in_=e1t[bi][:, kc * P:(kc + 1) * P],
                        identity=ident[:],
                    )
                eT = etpool.tile([P, n_k0 + n_k1, P], BF16, tag="eT",
                                 name=f"eT_{sg}")
                nc.scalar.copy(out=eT[:], in_=psT[:])
                return eT

            def stage_back(sg, eT):
                """matmuls + merge + out DMA"""
                bi = sg % NBUF
                Ws = Wt[bi]
                mm = psMMpool.tile([P, D], FP32, tag="mm", name=f"mm_{sg}")
                for h in range(2):
                    fs = slice(h * 512, (h + 1) * 512)
                    for kc in range(n_k0):
                        nc.tensor.matmul(
                            out=mm[:, fs], lhsT=eT[:, kc, :],
                            rhs=w0T[:, kc, fs],
                            start=(kc == 0), stop=False,
                        )
                    for kc in range(n_k1):
                        nc.tensor.matmul(
                            out=mm[:, fs], lhsT=eT[:, n_k0 + kc, :],
                            rhs=w1T[:, kc, fs],
                            start=False, stop=(kc == n_k1 - 1),
                        )
                v.copy_predicated(
                    out=Ws[:],
                    mask=ge5k[:, sg:sg + 1].to_broadcast([P, D]),
                    data=mm[:, :],
                )
                nc.sync.dma_start(out_r[:, sg, :], Ws[:])

            from collections import deque

            pending = deque()
            for sg in range(n_slots):
                eT = stage_front(sg)
                pending.append((sg, eT))
                if len(pending) > 2:
                    stage_back(*pending.popleft())
            while pending:
                stage_back(*pending.popleft())

    nc.compile()
    return nc


_GRAPH_CACHE = {}


def _get_graph(n_tok=4096, chunk_slots=8):
    key = (n_tok, chunk_slots)
    if key not in _GRAPH_CACHE:
        _GRAPH_CACHE[key] = build_graph(n_tok, chunk_slots)
    return _GRAPH_CACHE[key]


def make_tables(head_emb, tail0_emb, tail1_emb):
    flat = np.concatenate([
        np.ascontiguousarray(head_emb, dtype=np.float32).ravel(),
        np.ascontiguousarray(tail0_emb, dtype=np.float32).ravel(),
        np.ascontiguousarray(tail1_emb, dtype=np.float32).ravel(),
        np.zeros(3 * WROW, np.float32),
    ])
    return flat.reshape(TAB_ROWS, WROW)


def make_wT(w, h):
    """[D, h] f32 -> [128, h//128, D] bf16 with (p, kc, f) = w[f, kc*128+p]"""
    import ml_dtypes

    wt = np.ascontiguousarray(w, dtype=np.float32).T  # [h, D]
    wt = wt.reshape(h // P, P, D).transpose(1, 0, 2)  # [P, h//128, D]
    return np.ascontiguousarray(wt.astype(ml_dtypes.bfloat16))


def make_in_maps(tokens, head_emb, tail0_emb, tail0_w, tail1_emb, tail1_w):
    tables = make_tables(head_emb, tail0_emb, tail1_emb)
    w0T = make_wT(tail0_w, T0_H)
    w1T = make_wT(tail1_w, T1_H)
    return [
        {
            "tokens": np.ascontiguousarray(tokens[b].astype(np.int32).reshape(-1)),
            "tables": tables,
            "tail0_wT": w0T,
            "tail1_wT": w1T,
        }
        for b in range(tokens.shape[0])
    ]


def _ensure_axon_hooks():
    """bass_utils imports antenv.axon_hooks when tracing is requested via
    env; provide a no-op fallback module if the image lacks it."""
    import sys
    import types

    try:
        import antenv.axon_hooks  # noqa: F401
    except Exception:
        mod = types.ModuleType("antenv.axon_hooks")
        mod._hook = None
        mod.set_axon_ntff_profile_hook = lambda h: setattr(mod, "_hook", h)
        mod.get_axon_ntff_profile_hook = lambda: mod._hook
        sys.modules["antenv.axon_hooks"] = mod
        try:
            import antenv

            antenv.axon_hooks = mod
        except Exception:
            pass


def kernel(tokens, head_emb, tail0_emb, tail0_w, tail1_emb, tail1_w):
    _ensure_axon_hooks()
    from concourse.bass_utils import run_bass_kernel_spmd

    B, S = tokens.shape
    nc = _get_graph(n_tok=S, chunk_slots=8)
    in_maps = make_in_maps(tokens, head_emb, tail0_emb, tail0_w, tail1_emb, tail1_w)
    res = run_bass_kernel_spmd(nc, in_maps, core_ids=list(range(B)))
    out = np.stack([r["out"] for r in res.results], axis=0)
    return out.reshape(B, S, D).astype(np.float32)


# revision 28
# speedup vs baseline: 1.0224x; 1.0224x over previous
"""AdaptiveInput embedding lookup kernel for TRN2 (8 NeuronCores).

Strategy: pure data-parallel over tokens. tokens (8, 4096) -> one batch row
per core (4096 tokens each); embedding tables replicated to every core; no
collectives.

The three cluster tables are concatenated host-side into one flat f32 buffer
viewed as [80263, 256] (head rows start at window-row 0, tail0 rows at 20000,
tail1 at 50000, 3 zero rows of padding).  Every token gathers one uniform
4KB window starting at its table row, so a single indirect-DMA gather stream
serves all three clusters, and the window's first 1024|512|256 floats are
exactly the token's embedding row.

Per-core layout: token j sits at (partition p=j%128, slot s=j//128); 4 chunks
of 8 slots (1024 tokens).  Per chunk:
  * per-slot [128,1]-offset indirect_dma_start gathers windows into the
    output tile W [128, 8, 1024] f32
  * per slot: mask-multiply-cast W[:, s, :512]/[:, s, :256] to bf16 (zeroing
    rows of tokens from other clusters), PE-transpose 128x128 blocks to put
    the contraction dim on partitions, matmul against pre-transposed bf16
    weights into PSUM, then copy_predicated the projection into W for every
    non-head token (head rows keep the gathered embedding)
  * DMA W to HBM rows s*128+p
"""

import numpy as np

import concourse.bass as bass
import concourse.mybir as mybir
import concourse.tile as tile
from concourse import bacc
from concourse.bass import IndirectOffsetOnAxis
from concourse.masks import make_identity

FP32 = mybir.dt.float32
BF16 = mybir.dt.bfloat16
I32 = mybir.dt.int32
Alu = mybir.AluOpType

P = 128
D = 1024  # IN_FEATURES
HEAD_ROWS = 5000
T0_ROWS, T0_H = 15000, 512
T1_ROWS, T1_H = 30257, 256
CUT1, CUT2 = 5000, 20000

# flat concat of tables in 256-float windows rows
WROW = 256
T0_BASE = HEAD_ROWS * (D // WROW)            # 20000
T1_BASE = T0_BASE + T0_ROWS * (T0_H // WROW)  # 50000
TAB_ROWS = T1_BASE + T1_ROWS * (T1_H // WROW) + 3  # 80260 + 3 pad rows

N_CORES = 8


def build_graph(n_tok=4096, chunk_slots=8):
    n_slots = n_tok // P
    n_chunks = n_slots // chunk_slots
    assert n_chunks * chunk_slots == n_slots

    nc = bacc.Bacc("TRN2", target_bir_lowering=False, debug=False)

    tok_ext = nc.dram_tensor("tokens", [n_tok], I32, kind="ExternalInput")
    tab_ext = nc.dram_tensor("tables", [TAB_ROWS, WROW], FP32, kind="ExternalInput")
    t0wT_ext = nc.dram_tensor("tail0_wT", [P, T0_H // P, D], BF16,
                              kind="ExternalInput")
    t1wT_ext = nc.dram_tensor("tail1_wT", [P, T1_H // P, D], BF16,
                              kind="ExternalInput")
    out_ext = nc.dram_tensor("out", [n_tok, D], FP32, kind="ExternalOutput")
    out_r = out_ext.rearrange("(p s) d -> p s d", p=P)  # row p*n_slots+s

    with tile.TileContext(nc) as tc:
        with (
            tc.tile_pool(name="const", bufs=1) as cpool,
            tc.tile_pool(name="work", bufs=1) as wpool,
            tc.tile_pool(name="eT", bufs=3) as etpool,
            tc.tile_pool(name="psT0", bufs=3, space="PSUM") as psT0pool,
            tc.tile_pool(name="psMM", bufs=2, space="PSUM") as psMMpool,
        ):
            # ---- token-derived window indices and masks (first: the
            # gathers depend on these) ----
            # token j at (p=j//n_slots, s=j%n_slots): contiguous DMA
            tok_m = cpool.tile([P, n_slots], I32, tag="tok_m")
            nc.sync.dma_start(tok_m[:], tok_ext.rearrange("(p s) -> p s", p=P))

            v = nc.vector

            ge5k = cpool.tile([P, n_slots], I32, tag="ge5k")
            tmp = cpool.tile([P, n_slots], I32, tag="tmpi")
            tmp2 = cpool.tile([P, n_slots], I32, tag="tmpi2")
            widx = cpool.tile([P, n_slots], I32, tag="widx")
            m1b = cpool.tile([P, n_slots], BF16, tag="m1b")
            m2b = cpool.tile([P, n_slots], BF16, tag="m2b")

            v.tensor_scalar(ge5k[:], tok_m[:], CUT1, None, op0=Alu.is_ge)
            # window row index:
            #   t<5k: 4t ; 5k<=t<20k: 2t+10000 ; t>=20k: t+30000
            # = 4t - ge5k*(2t-10000) - ge20k*(t-20000)
            v.tensor_scalar(tmp[:], tok_m[:], 2, 10000, op0=Alu.mult, op1=Alu.subtract)
            v.tensor_tensor(tmp[:], tmp[:], ge5k[:], op=Alu.mult)
            v.tensor_scalar(widx[:], tok_m[:], 4, None, op0=Alu.mult)
            v.tensor_tensor(widx[:], widx[:], tmp[:], op=Alu.subtract)
            v.tensor_scalar(tmp2[:], tok_m[:], CUT2, None, op0=Alu.subtract)
            v.tensor_scalar(tmp[:], tok_m[:], CUT2, None, op0=Alu.is_ge)
            v.tensor_copy(m2b[:], tmp[:])
            v.tensor_tensor(tmp2[:], tmp2[:], tmp[:], op=Alu.mult)
            v.tensor_tensor(widx[:], widx[:], tmp2[:], op=Alu.subtract)
            # masks: m1 = (t>=5000)&(t<20000) ; m2 = t>=20000
            v.tensor_scalar(tmp[:], tok_m[:], CUT2, None, op0=Alu.is_lt)
            v.tensor_tensor(tmp[:], tmp[:], ge5k[:], op=Alu.mult)
            v.tensor_copy(m1b[:], tmp[:])
            # head top-up window index: 4t+2 for head tokens, OOB otherwise
            idx_top = cpool.tile([P, n_slots], I32, tag="idx_top")
            v.tensor_scalar(tmp[:], ge5k[:], 100000, None, op0=Alu.mult)
            v.tensor_scalar(idx_top[:], tok_m[:], 4, 2, op0=Alu.mult, op1=Alu.add)
            v.tensor_tensor(idx_top[:], idx_top[:], tmp[:], op=Alu.add)

            # ---- constants / one-time prep ----
            ident = cpool.tile([P, P], BF16, tag="ident")
            make_identity(nc, ident[:])

            # HAM warm-up: dependency-free matmuls so the PE clock reaches
            # 8/8 before the first real transposes/matmuls arrive
            warm = cpool.tile([P, 512], BF16, tag="warm")
            nc.vector.memset(warm[:], 0.0)
            wps = psMMpool.tile([P, D], FP32, tag="mm", name="warmps")
            for _ in range(16):
                nc.tensor.matmul(out=wps[:, 0:512], lhsT=warm[:, 0:P], rhs=warm[:],
                                 start=True, stop=True)

            # weights arrive pre-transposed/pre-cast: [k%128, k//128, f] bf16
            n_k0 = T0_H // P  # 4
            n_k1 = T1_H // P  # 2
            w0T = cpool.tile([P, n_k0, D], BF16, tag="w0T")
            w1T = cpool.tile([P, n_k1, D], BF16, tag="w1T")
            nc.sync.dma_start(w0T[:], t0wT_ext[:, :, :])
            nc.sync.dma_start(w1T[:], t1wT_ext[:, :, :])

            # ---- main loop: flat slot stream, software-pipelined so the
            # PE runs transposes(s) back-to-back with matmuls(s-1) while the
            # ACT copy of slot s proceeds in parallel ----
            NPAIR = 12  # pair tiles in flight (gathers run ahead)
            Wt = [wpool.tile([P, 2, D], FP32, tag=f"W_{i}", name=f"W_{i}")
                  for i in range(NPAIR)]
            NBUF = 2 * NPAIR
            e0t = [wpool.tile([P, T0_H], BF16, tag=f"e0b_{i}", name=f"e0b_{i}")
                   for i in range(NBUF)]
            e1t = [wpool.tile([P, T1_H], BF16, tag=f"e1b_{i}", name=f"e1b_{i}")
                   for i in range(NBUF)]

            def stage_front(sg):
                """gather + mask/cast + transpose + psum->sbuf copy"""
                bi = sg % NBUF
                Ws = Wt[(sg // 2) % NPAIR][:, sg % 2, :]
                nc.gpsimd.indirect_dma_start(
                    out=Ws[:], out_offset=None,
                    in_=tab_ext[:, :],
                    in_offset=IndirectOffsetOnAxis(ap=widx[:, sg:sg + 1], axis=0),
                )
                v.tensor_tensor(
                    e0t[bi][:], Ws[:, 0:T0_H],
                    m1b[:, sg:sg + 1].to_broadcast([P, T0_H]), op=Alu.mult,
                )
                v.tensor_tensor(
                    e1t[bi][:], Ws[:, 0:T1_H],
                    m2b[:, sg:sg + 1].to_broadcast([P, T1_H]), op=Alu.mult,
                )
                psT = psT0pool.tile([P, n_k0 + n_k1, P], BF16, tag="psw",
                                    name=f"psT_{sg}")
                for kc in range(n_k0):
                    nc.tensor.transpose(
                        out=psT[:, kc, :],
                        in_=e0t[bi][:, kc * P:(kc + 1) * P],
                        identity=ident[:],
                    )
                for kc in range(n_k1):
                    nc.tensor.transpose(
                        out=psT[:, n_k0 + kc, :],
                        in_=e1t[bi][:, kc * P:(kc + 1) * P],
                        identity=ident[:],
                    )
                eT = etpool.tile([P, n_k0 + n_k1, P], BF16, tag="eT",
                                 name=f"eT_{sg}")
                nc.scalar.copy(out=eT[:], in_=psT[:])
                return eT

            def stage_back(sg, eT):
                """matmuls + merge + paired out DMA (8KB runs per partition)"""
                bi = sg % NBUF
                Ws = Wt[(sg // 2) % NPAIR][:, sg % 2, :]
                mm = psMMpool.tile([P, D], FP32, tag="mm", name=f"mm_{sg}")
                for h in range(2):
                    fs = slice(h * 512, (h + 1) * 512)
                    for kc in range(n_k0):
                        nc.tensor.matmul(
                            out=mm[:, fs], lhsT=eT[:, kc, :],
                            rhs=w0T[:, kc, fs],
                            start=(kc == 0), stop=False,
                        )
                    for kc in range(n_k1):
                        nc.tensor.matmul(
                            out=mm[:, fs], lhsT=eT[:, n_k0 + kc, :],
                            rhs=w1T[:, kc, fs],
                            start=False, stop=(kc == n_k1 - 1),
                        )
                v.copy_predicated(
                    out=Ws[:],
                    mask=ge5k[:, sg:sg + 1].to_broadcast([P, D]),
                    data=mm[:, :],
                )
                if sg % 2 == 1:
                    nc.sync.dma_start(
                        out_r[:, sg - 1:sg + 1, :],
                        Wt[(sg // 2) % NPAIR][:, :, :],
                    )

            pending = None
            for sg in range(n_slots):
                eT = stage_front(sg)
                if pending is not None:
                    stage_back(*pending)
                pending = (sg, eT)
            stage_back(*pending)

    nc.compile()
    return nc


_GRAPH_CACHE = {}


def _get_graph(n_tok=4096, chunk_slots=8):
    key = (n_tok, chunk_slots)
    if key not in _GRAPH_CACHE:
        _GRAPH_CACHE[key] = build_graph(n_tok, chunk_slots)
    return _GRAPH_CACHE[key]


def make_tables(head_emb, tail0_emb, tail1_emb):
    flat = np.concatenate([
        np.ascontiguousarray(head_emb, dtype=np.float32).ravel(),
        np.ascontiguousarray(tail0_emb, dtype=np.float32).ravel(),
        np.ascontiguousarray(tail1_emb, dtype=np.float32).ravel(),
        np.zeros(3 * WROW, np.float32),
    ])
    return flat.reshape(TAB_ROWS, WROW)


def make_wT(w, h):
    """[D, h] f32 -> [128, h//128, D] bf16 with (p, kc, f) = w[f, kc*128+p]"""
    import ml_dtypes

    wt = np.ascontiguousarray(w, dtype=np.float32).T  # [h, D]
    wt = wt.reshape(h // P, P, D).transpose(1, 0, 2)  # [P, h//128, D]
    return np.ascontiguousarray(wt.astype(ml_dtypes.bfloat16))


def make_in_maps(tokens, head_emb, tail0_emb, tail0_w, tail1_emb, tail1_w):
    tables = make_tables(head_emb, tail0_emb, tail1_emb)
    w0T = make_wT(tail0_w, T0_H)
    w1T = make_wT(tail1_w, T1_H)
    return [
        {
            "tokens": np.ascontiguousarray(tokens[b].astype(np.int32).reshape(-1)),
            "tables": tables,
            "tail0_wT": w0T,
            "tail1_wT": w1T,
        }
        for b in range(tokens.shape[0])
    ]


def _ensure_axon_hooks():
    """bass_utils imports antenv.axon_hooks when tracing is requested via
    env; provide a no-op fallback module if the image lacks it."""
    import sys
    import types

    try:
        import antenv.axon_hooks  # noqa: F401
    except Exception:
        mod = types.ModuleType("antenv.axon_hooks")
        mod._hook = None
        mod.set_axon_ntff_profile_hook = lambda h: setattr(mod, "_hook", h)
        mod.get_axon_ntff_profile_hook = lambda: mod._hook
        sys.modules["antenv.axon_hooks"] = mod
        try:
            import antenv

            antenv.axon_hooks = mod
        except Exception:
            pass


def kernel(tokens, head_emb, tail0_emb, tail0_w, tail1_emb, tail1_w):
    _ensure_axon_hooks()
    from concourse.bass_utils import run_bass_kernel_spmd

    B, S = tokens.shape
    nc = _get_graph(n_tok=S, chunk_slots=8)
    in_maps = make_in_maps(tokens, head_emb, tail0_emb, tail0_w, tail1_emb, tail1_w)
    res = run_bass_kernel_spmd(nc, in_maps, core_ids=list(range(B)))
    out = np.stack([r["out"] for r in res.results], axis=0)
    return out.reshape(B, S, D).astype(np.float32)


# revision 29
# speedup vs baseline: 1.0826x; 1.0589x over previous
"""AdaptiveInput embedding lookup kernel for TRN2 (8 NeuronCores).

Strategy: pure data-parallel over tokens. tokens (8, 4096) -> one batch row
per core (4096 tokens each); embedding tables replicated to every core; no
collectives.

The three cluster tables are concatenated host-side into one flat f32 buffer
viewed as [80263, 256] (head rows start at window-row 0, tail0 rows at 20000,
tail1 at 50000, 3 zero rows of padding).  Every token gathers one uniform
4KB window starting at its table row, so a single indirect-DMA gather stream
serves all three clusters, and the window's first 1024|512|256 floats are
exactly the token's embedding row.

Per-core layout: token j sits at (partition p=j%128, slot s=j//128); 4 chunks
of 8 slots (1024 tokens).  Per chunk:
  * per-slot [128,1]-offset indirect_dma_start gathers windows into the
    output tile W [128, 8, 1024] f32
  * per slot: mask-multiply-cast W[:, s, :512]/[:, s, :256] to bf16 (zeroing
    rows of tokens from other clusters), PE-transpose 128x128 blocks to put
    the contraction dim on partitions, matmul against pre-transposed bf16
    weights into PSUM, then copy_predicated the projection into W for every
    non-head token (head rows keep the gathered embedding)
  * DMA W to HBM rows s*128+p
"""

import numpy as np

import concourse.bass as bass
import concourse.mybir as mybir
import concourse.tile as tile
from concourse import bacc
from concourse.bass import IndirectOffsetOnAxis
from concourse.masks import make_identity

FP32 = mybir.dt.float32
BF16 = mybir.dt.bfloat16
I32 = mybir.dt.int32
Alu = mybir.AluOpType

P = 128
D = 1024  # IN_FEATURES
HEAD_ROWS = 5000
T0_ROWS, T0_H = 15000, 512
T1_ROWS, T1_H = 30257, 256
CUT1, CUT2 = 5000, 20000

# flat concat of tables in 256-float windows rows
WROW = 256
T0_BASE = HEAD_ROWS * (D // WROW)            # 20000
T1_BASE = T0_BASE + T0_ROWS * (T0_H // WROW)  # 50000
TAB_ROWS = T1_BASE + T1_ROWS * (T1_H // WROW) + 3  # 80260 + 3 pad rows

N_CORES = 8


def build_graph(n_tok=4096, chunk_slots=8):
    n_slots = n_tok // P
    n_chunks = n_slots // chunk_slots
    assert n_chunks * chunk_slots == n_slots

    nc = bacc.Bacc("TRN2", target_bir_lowering=False, debug=False)

    tok_ext = nc.dram_tensor("tokens", [n_tok], I32, kind="ExternalInput")
    tab_ext = nc.dram_tensor("tables", [TAB_ROWS, WROW], FP32, kind="ExternalInput")
    t0wT_ext = nc.dram_tensor("tail0_wT", [P, T0_H // P, D], BF16,
                              kind="ExternalInput")
    t1wT_ext = nc.dram_tensor("tail1_wT", [P, T1_H // P, D], BF16,
                              kind="ExternalInput")
    out_ext = nc.dram_tensor("out", [n_tok, D], FP32, kind="ExternalOutput")
    out_r = out_ext.rearrange("(p s) d -> p s d", p=P)  # row p*n_slots+s

    with tile.TileContext(nc) as tc:
        with (
            tc.tile_pool(name="const", bufs=1) as cpool,
            tc.tile_pool(name="work", bufs=1) as wpool,
            tc.tile_pool(name="eT", bufs=3) as etpool,
            tc.tile_pool(name="psT0", bufs=3, space="PSUM") as psT0pool,
            tc.tile_pool(name="psMM", bufs=2, space="PSUM") as psMMpool,
        ):
            # ---- token-derived window indices and masks (first: the
            # gathers depend on these) ----
            # token j at (p=j//n_slots, s=j%n_slots): contiguous DMA
            tok_m = cpool.tile([P, n_slots], I32, tag="tok_m")
            nc.sync.dma_start(tok_m[:], tok_ext.rearrange("(p s) -> p s", p=P))

            v = nc.vector

            ge5k = cpool.tile([P, n_slots], I32, tag="ge5k")
            tmp = cpool.tile([P, n_slots], I32, tag="tmpi")
            tmp2 = cpool.tile([P, n_slots], I32, tag="tmpi2")
            widx = cpool.tile([P, n_slots], I32, tag="widx")
            m1b = cpool.tile([P, n_slots], BF16, tag="m1b")
            m2b = cpool.tile([P, n_slots], BF16, tag="m2b")

            v.tensor_scalar(ge5k[:], tok_m[:], CUT1, None, op0=Alu.is_ge)
            # window row index:
            #   t<5k: 4t ; 5k<=t<20k: 2t+10000 ; t>=20k: t+30000
            # = 4t - ge5k*(2t-10000) - ge20k*(t-20000)
            v.tensor_scalar(tmp[:], tok_m[:], 2, 10000, op0=Alu.mult, op1=Alu.subtract)
            v.tensor_tensor(tmp[:], tmp[:], ge5k[:], op=Alu.mult)
            v.tensor_scalar(widx[:], tok_m[:], 4, None, op0=Alu.mult)
            v.tensor_tensor(widx[:], widx[:], tmp[:], op=Alu.subtract)
            v.tensor_scalar(tmp2[:], tok_m[:], CUT2, None, op0=Alu.subtract)
            v.tensor_scalar(tmp[:], tok_m[:], CUT2, None, op0=Alu.is_ge)
            v.tensor_copy(m2b[:], tmp[:])
            v.tensor_tensor(tmp2[:], tmp2[:], tmp[:], op=Alu.mult)
            v.tensor_tensor(widx[:], widx[:], tmp2[:], op=Alu.subtract)
            # masks: m1 = (t>=5000)&(t<20000) ; m2 = t>=20000
            v.tensor_scalar(tmp[:], tok_m[:], CUT2, None, op0=Alu.is_lt)
            v.tensor_tensor(tmp[:], tmp[:], ge5k[:], op=Alu.mult)
            v.tensor_copy(m1b[:], tmp[:])
            # head top-up window index: 4t+2 for head tokens, OOB otherwise
            idx_top = cpool.tile([P, n_slots], I32, tag="idx_top")
            v.tensor_scalar(tmp[:], ge5k[:], 100000, None, op0=Alu.mult)
            v.tensor_scalar(idx_top[:], tok_m[:], 4, 2, op0=Alu.mult, op1=Alu.add)
            v.tensor_tensor(idx_top[:], idx_top[:], tmp[:], op=Alu.add)

            # ---- constants / one-time prep ----
            ident = cpool.tile([P, P], BF16, tag="ident")
            make_identity(nc, ident[:])

            # HAM warm-up: dependency-free matmuls so the PE clock reaches
            # 8/8 before the first real transposes/matmuls arrive
            warm = cpool.tile([P, 512], BF16, tag="warm")
            nc.vector.memset(warm[:], 0.0)
            wps = psMMpool.tile([P, D], FP32, tag="mm", name="warmps")
            for _ in range(16):
                nc.tensor.matmul(out=wps[:, 0:512], lhsT=warm[:, 0:P], rhs=warm[:],
                                 start=True, stop=True)

            # weights arrive pre-transposed/pre-cast: [k%128, k//128, f] bf16
            n_k0 = T0_H // P  # 4
            n_k1 = T1_H // P  # 2
            w0T = cpool.tile([P, n_k0, D], BF16, tag="w0T")
            w1T = cpool.tile([P, n_k1, D], BF16, tag="w1T")
            nc.sync.dma_start(w0T[:], t0wT_ext[:, :, :])
            nc.sync.dma_start(w1T[:], t1wT_ext[:, :, :])

            # ---- main loop: flat slot stream, software-pipelined so the
            # PE runs transposes(s) back-to-back with matmuls(s-1) while the
            # ACT copy of slot s proceeds in parallel ----
            NBUF = 24  # slot tiles in flight (gathers run ahead)
            Wt = [wpool.tile([P, D], FP32, tag=f"W_{i}", name=f"W_{i}")
                  for i in range(NBUF)]
            e0t = [wpool.tile([P, T0_H], BF16, tag=f"e0b_{i}", name=f"e0b_{i}")
                   for i in range(NBUF)]
            e1t = [wpool.tile([P, T1_H], BF16, tag=f"e1b_{i}", name=f"e1b_{i}")
                   for i in range(NBUF)]

            def stage_front(sg):
                """gather + mask/cast + transpose + psum->sbuf copy"""
                bi = sg % NBUF
                Ws = Wt[bi]
                nc.gpsimd.indirect_dma_start(
                    out=Ws[:], out_offset=None,
                    in_=tab_ext[:, :],
                    in_offset=IndirectOffsetOnAxis(ap=widx[:, sg:sg + 1], axis=0),
                )
                v.tensor_tensor(
                    e0t[bi][:], Ws[:, 0:T0_H],
                    m1b[:, sg:sg + 1].to_broadcast([P, T0_H]), op=Alu.mult,
                )
                v.tensor_tensor(
                    e1t[bi][:], Ws[:, 0:T1_H],
                    m2b[:, sg:sg + 1].to_broadcast([P, T1_H]), op=Alu.mult,
                )
                psT = psT0pool.tile([P, n_k0 + n_k1, P], BF16, tag="psw",
                                    name=f"psT_{sg}")
                for kc in range(n_k0):
                    nc.tensor.transpose(
                        out=psT[:, kc, :],
                        in_=e0t[bi][:, kc * P:(kc + 1) * P],
                        identity=ident[:],
                    )
                for kc in range(n_k1):
                    nc.tensor.transpose(
                        out=psT[:, n_k0 + kc, :],
                        in_=e1t[bi][:, kc * P:(kc + 1) * P],
                        identity=ident[:],
                    )
                eT = etpool.tile([P, n_k0 + n_k1, P], BF16, tag="eT",
                                 name=f"eT_{sg}")
                nc.scalar.copy(out=eT[:], in_=psT[:])
                return eT

            def stage_back(sg, eT):
                """matmuls + merge + out DMA"""
                bi = sg % NBUF
                Ws = Wt[bi]
                mm = psMMpool.tile([P, D], FP32, tag="mm", name=f"mm_{sg}")
                for h in range(2):
                    fs = slice(h * 512, (h + 1) * 512)
                    for kc in range(n_k0):
                        nc.tensor.matmul(
                            out=mm[:, fs], lhsT=eT[:, kc, :],
                            rhs=w0T[:, kc, fs],
                            start=(kc == 0), stop=False,
                        )
                    for kc in range(n_k1):
                        nc.tensor.matmul(
                            out=mm[:, fs], lhsT=eT[:, n_k0 + kc, :],
                            rhs=w1T[:, kc, fs],
                            start=False, stop=(kc == n_k1 - 1),
                        )
                v.copy_predicated(
                    out=Ws[:],
                    mask=ge5k[:, sg:sg + 1].to_broadcast([P, D]),
                    data=mm[:, :],
                )
                nc.sync.dma_start(out_r[:, sg, :], Ws[:])

            pending = None
            for sg in range(n_slots):
                eT = stage_front(sg)
                if pending is not None:
                    stage_back(*pending)
                pending = (sg, eT)
            stage_back(*pending)

    nc.compile()
    return nc


_GRAPH_CACHE = {}


def _get_graph(n_tok=4096, chunk_slots=8):
    key = (n_tok, chunk_slots)
    if key not in _GRAPH_CACHE:
        _GRAPH_CACHE[key] = build_graph(n_tok, chunk_slots)
    return _GRAPH_CACHE[key]


def make_tables(head_emb, tail0_emb, tail1_emb):
    flat = np.concatenate([
        np.ascontiguousarray(head_emb, dtype=np.float32).ravel(),
        np.ascontiguousarray(tail0_emb, dtype=np.float32).ravel(),
        np.ascontiguousarray(tail1_emb, dtype=np.float32).ravel(),
        np.zeros(3 * WROW, np.float32),
    ])
    return flat.reshape(TAB_ROWS, WROW)


def make_wT(w, h):
    """[D, h] f32 -> [128, h//128, D] bf16 with (p, kc, f) = w[f, kc*128+p]"""
    import ml_dtypes

    wt = np.ascontiguousarray(w, dtype=np.float32).T  # [h, D]
    wt = wt.reshape(h // P, P, D).transpose(1, 0, 2)  # [P, h//128, D]
    return np.ascontiguousarray(wt.astype(ml_dtypes.bfloat16))


def make_in_maps(tokens, head_emb, tail0_emb, tail0_w, tail1_emb, tail1_w):
    tables = make_tables(head_emb, tail0_emb, tail1_emb)
    w0T = make_wT(tail0_w, T0_H)
    w1T = make_wT(tail1_w, T1_H)
    return [
        {
            "tokens": np.ascontiguousarray(tokens[b].astype(np.int32).reshape(-1)),
            "tables": tables,
            "tail0_wT": w0T,
            "tail1_wT": w1T,
        }
        for b in range(tokens.shape[0])
    ]


def _ensure_axon_hooks():
    """bass_utils imports antenv.axon_hooks when tracing is requested via
    env; provide a no-op fallback module if the image lacks it."""
    import sys
    import types

    try:
        import antenv.axon_hooks  # noqa: F401
    except Exception:
        mod = types.ModuleType("antenv.axon_hooks")
        mod._hook = None
        mod.set_axon_ntff_profile_hook = lambda h: setattr(mod, "_hook", h)
        mod.get_axon_ntff_profile_hook = lambda: mod._hook
        sys.modules["antenv.axon_hooks"] = mod
        try:
            import antenv

            antenv.axon_hooks = mod
        except Exception:
            pass


def kernel(tokens, head_emb, tail0_emb, tail0_w, tail1_emb, tail1_w):
    _ensure_axon_hooks()
    from concourse.bass_utils import run_bass_kernel_spmd

    B, S = tokens.shape
    nc = _get_graph(n_tok=S, chunk_slots=8)
    in_maps = make_in_maps(tokens, head_emb, tail0_emb, tail0_w, tail1_emb, tail1_w)
    res = run_bass_kernel_spmd(nc, in_maps, core_ids=list(range(B)))
    out = np.stack([r["out"] for r in res.results], axis=0)
    return out.reshape(B, S, D).astype(np.float32)
